# revision 10
# baseline (speedup 1.0000x reference)
"""CQCNN piece estimator on 8 trn2 NeuronCores.

Strategy: pure data parallel over batch (8192 samples/core), SPMD (one NEFF).
Activations live feature-major [features(partitions), batch(free)].
Convs on the 6x6/3x3 boards are dense linear maps -> matmuls with
zero-block skipping.  Maxpool via DMA parity-gather + 3 tensor_max.
Quantum sim runs qubit-interleaved on 128 partitions.  softmax(2)/sigmoid
are rewritten in terms of tanh so one ACT table set serves everything.
"""

import numpy as np
import ml_dtypes

import concourse.bass as bass
import concourse.bacc as bacc
import concourse.mybir as mybir
import concourse.tile as tile
from concourse.bass_utils import run_bass_kernel_spmd

BF16 = mybir.dt.bfloat16
F32 = mybir.dt.float32
nbf = ml_dtypes.bfloat16

B = 65536
NCORES = 8
BC = B // NCORES          # 8192 per core
CB = 2048                 # chunk of batch processed per pipeline pass
NCHUNK = BC // CB         # 4
NSL = 512                 # matmul moving-operand slice
NSLC = CB // NSL          # 4 slices per chunk
NQ = 8

AF = mybir.ActivationFunctionType
ALU = mybir.AluOpType

_cache = {}


def _build_conv_maps(conv1_w, conv2_w, conv3_w):
    """Dense linear maps for the three convs, with my feature orderings.

    X in-features  : channel-major c*36 + y*6 + x   (== board reshape order)
    H1 out-features: pos-major (y*6+x)*16 + c
    H2 out-features: pos-major (y*6+x)*32 + c
    P  (pooled)    : pos-major (y*3+x)*32 + c
    H3 out-features: pos-major (y*3+x)*64 + c
    """
    T1 = np.zeros((108, 576), np.float32)
    for co in range(16):
        for ci in range(3):
            for ky in range(3):
                for kx in range(3):
                    w = conv1_w[co, ci, ky, kx]
                    for yo in range(6):
                        yi = yo + ky - 1
                        if not 0 <= yi < 6:
                            continue
                        for xo in range(6):
                            xi = xo + kx - 1
                            if 0 <= xi < 6:
                                T1[ci * 36 + yi * 6 + xi, (yo * 6 + xo) * 16 + co] = w

    T2 = np.zeros((576, 1152), np.float32)
    for ky in range(3):
        for kx in range(3):
            w = conv2_w[:, :, ky, kx]  # [32,16]
            for yo in range(6):
                yi = yo + ky - 1
                if not 0 <= yi < 6:
                    continue
                for xo in range(6):
                    xi = xo + kx - 1
                    if 0 <= xi < 6:
                        pi, po = (yi * 6 + xi) * 16, (yo * 6 + xo) * 32
                        T2[pi:pi + 16, po:po + 32] = w.T

    T3 = np.zeros((288, 576), np.float32)
    for ky in range(3):
        for kx in range(3):
            w = conv3_w[:, :, ky, kx]  # [64,32]
            for yo in range(3):
                yi = yo + ky - 1
                if not 0 <= yi < 3:
                    continue
                for xo in range(3):
                    xi = xo + kx - 1
                    if 0 <= xi < 3:
                        pi, po = (yi * 3 + xi) * 32, (yo * 3 + xo) * 64
                        T3[pi:pi + 32, po:po + 64] = w.T
    return T1, T2, T3


def _parts(n, step=128):
    return [(i, min(i + step, n)) for i in range(0, n, step)]


def _nonzero_blocks(T, kparts, mparts):
    out = {}
    for mj, (m0, m1) in enumerate(mparts):
        ks = [ki for ki, (k0, k1) in enumerate(kparts)
              if np.any(T[k0:k1, m0:m1])]
        out[mj] = ks
    return out


def _build_program():
    nc = bacc.Bacc("TRN2", target_bir_lowering=False, debug=False)

    xT_d = nc.dram_tensor("xT", [108, BC], BF16, kind="ExternalInput")
    qx_d = nc.dram_tensor("qx", [128, BC // 16], F32, kind="ExternalInput")
    qxn_d = nc.dram_tensor("qxn", [128, BC // 16], F32, kind="ExternalInput")
    t1_d = nc.dram_tensor("t1d", [108, 576], BF16, kind="ExternalInput")
    t2_d = nc.dram_tensor("t2d", [576, 1152], BF16, kind="ExternalInput")
    t3_d = nc.dram_tensor("t3d", [288, 576], BF16, kind="ExternalInput")
    w1_d = nc.dram_tensor("w1d", [584, 192], BF16, kind="ExternalInput")
    w2_d = nc.dram_tensor("w2pd", [128, 64], BF16, kind="ExternalInput")
    w3_d = nc.dram_tensor("w3cd", [128, 3], BF16, kind="ExternalInput")
    s_d = nc.dram_tensor("Sd", [3, 3], F32, kind="ExternalInput")
    b1_d = nc.dram_tensor("b1t", [128, 5], F32, kind="ExternalInput")
    b2_d = nc.dram_tensor("b2t", [128, 9], F32, kind="ExternalInput")
    b3_d = nc.dram_tensor("b3t", [128, 5], F32, kind="ExternalInput")
    bm1_d = nc.dram_tensor("bm1", [128, 2], F32, kind="ExternalInput")
    bm2_d = nc.dram_tensor("bm2", [128, 1], F32, kind="ExternalInput")
    bh_d = nc.dram_tensor("bh", [3, 1], F32, kind="ExternalInput")
    rot_d = nc.dram_tensor("rot", [128, 9], F32, kind="ExternalInput")
    out_d = nc.dram_tensor("out", [3, BC], F32, kind="ExternalOutput")

    k1 = _parts(108)          # 1 part
    m1p = _parts(576)         # 5
    k2 = _parts(576)          # 5
    m2p = _parts(1152)        # 9
    k3 = _parts(288)          # 3  (128,128,32)
    m3p = _parts(576)         # 5
    km = _parts(584)          # 5  (last 72 = conv tail 64 + quantum 8)

    blocks2 = _cache["blocks2"]
    blocks3 = _cache["blocks3"]

    # parity gather map: H2 position -> (parity, pooled-pos)
    # src H2 tile pos//4 rows (pos%4)*32 ; dst parity tile q//4 rows (q%4)*32
    gather = []  # (src_tile, src_row, parity, dst_tile, dst_row)
    for y in range(6):
        for x in range(6):
            pos = y * 6 + x
            par = (y % 2) * 2 + (x % 2)
            q = (y // 2) * 3 + (x // 2)
            gather.append((pos // 4, (pos % 4) * 32, par, q // 4, (q % 4) * 32))

    from contextlib import ExitStack
    with tile.TileContext(nc) as tc, ExitStack() as ctx:
        wts = ctx.enter_context(tc.tile_pool(name="wts", bufs=1))
        qp = ctx.enter_context(tc.tile_pool(name="qp", bufs=1))
        xp = ctx.enter_context(tc.tile_pool(name="xp", bufs=2))
        h1p = ctx.enter_context(tc.tile_pool(name="h1p", bufs=1))
        h2p = ctx.enter_context(tc.tile_pool(name="h2p", bufs=1))
        prp = ctx.enter_context(tc.tile_pool(name="prp", bufs=1))
        pp = ctx.enter_context(tc.tile_pool(name="pp", bufs=1))
        h3p = ctx.enter_context(tc.tile_pool(name="h3p", bufs=1))
        hdp = ctx.enter_context(tc.tile_pool(name="hdp", bufs=1))
        psp = ctx.enter_context(tc.tile_pool(name="psp", bufs=6, space="PSUM"))
        psh = ctx.enter_context(tc.tile_pool(name="psh", bufs=2, space="PSUM"))

        nwt = [0]

        def dmaw(shape, dt, src):
            nwt[0] += 1
            t = wts.tile(shape, dt, tag=f"w{nwt[0]}", name=f"w{nwt[0]}")
            nc.gpsimd.dma_start(out=t, in_=src)
            return t

        t1 = dmaw([108, 576], BF16, t1_d[:, :])
        t2s = [dmaw([k1_ - k0_, 1152], BF16, t2_d[k0_:k1_, :]) for k0_, k1_ in k2]
        t3s = [dmaw([k1_ - k0_, 576], BF16, t3_d[k0_:k1_, :]) for k0_, k1_ in k3]
        w1s = [dmaw([k1_ - k0_, 192], BF16, w1_d[k0_:k1_, :]) for k0_, k1_ in km]
        w2 = dmaw([128, 64], BF16, w2_d[:, :])
        w3 = dmaw([128, 3], BF16, w3_d[:, :])
        smat = dmaw([3, 3], F32, s_d[:, :])
        b1 = dmaw([128, 5], F32, b1_d[:, :])
        b2 = dmaw([128, 9], F32, b2_d[:, :])
        b3 = dmaw([128, 5], F32, b3_d[:, :])
        bm1 = dmaw([128, 2], F32, bm1_d[:, :])
        bm2 = dmaw([128, 1], F32, bm2_d[:, :])
        bh = dmaw([3, 1], F32, bh_d[:, :])
        rot = dmaw([128, 9], F32, rot_d[:, :])

        zc = wts.tile([128, 1], F32, tag="zc", name="zc")
        nc.vector.memset(zc, 0.0)
        halfpi = wts.tile([128, 1], F32, tag="halfpi", name="halfpi")
        nc.vector.memset(halfpi, float(np.pi / 2))

        # ---- quantum sim, qubit-interleaved [q + 8g, j], b = g*512 + j ----
        qx = dmaw([128, BC // 16], F32, qx_d[:, :])
        qxn = dmaw([128, BC // 16], F32, qxn_d[:, :])
        qst = None
        for l in range(3):
            sa = qp.tile([128, BC // 16], F32, tag="sa", name=f"sa{l}")
            ca = qp.tile([128, BC // 16], F32, tag="ca", name=f"ca{l}")
            nc.vector.tensor_scalar_mul(sa, qx, rot[:, 3 * l:3 * l + 1])
            nc.vector.tensor_scalar_mul(ca, qxn, rot[:, 3 * l + 1:3 * l + 2])
            nc.scalar.activation(sa, sa, AF.Sin, bias=zc)
            nc.scalar.activation(ca, ca, AF.Sin, bias=halfpi)
            sc = qp.tile([128, BC // 16], F32, tag="sc", name=f"sc{l}")
            nc.vector.tensor_mul(sc, sa, ca)
            if qst is None:
                qst = sc
            else:
                ta = qp.tile([128, BC // 16], F32, tag="ta", name=f"ta{l}")
                nc.vector.tensor_scalar_mul(ta, qst, rot[:, 3 * l + 2:3 * l + 3])
                nc.scalar.activation(ta, ta, AF.Tanh, bias=zc)
                qn = qp.tile([128, BC // 16], F32, tag="qn", name=f"qn{l}")
                nc.vector.tensor_add(qn, sc, ta)
                qst = qn
        qfb = qp.tile([128, BC // 16], BF16, tag="qfb", name="qfb")
        nc.vector.tensor_copy(qfb, qst)

        # ---- main pipeline over batch chunks ----
        for c in range(NCHUNK):
            c0 = c * CB

            xc = xp.tile([108, CB], BF16, tag="xc", name="xc")
            nc.sync.dma_start(out=xc, in_=xT_d[:, c0:c0 + CB])

            h1 = [h1p.tile([m1_ - m0_, CB], BF16, tag=f"h1_{i}", name=f"h1_{i}")
                  for i, (m0_, m1_) in enumerate(m1p)]
            h2 = [h2p.tile([m1_ - m0_, CB], BF16, tag=f"h2_{i}", name=f"h2_{i}")
                  for i, (m0_, m1_) in enumerate(m2p)]
            pool = [pp.tile([r, CB], BF16, tag=f"pool_{t}", name=f"pool_{t}")
                    for t, r in enumerate((128, 128, 32))]
            h3 = [h3p.tile([128, CB], BF16, tag=f"h3_{i}", name=f"h3_{i}") for i in range(4)]
            h3t4 = h3p.tile([72, CB], BF16, tag="h3t4", name="h3t4")
            for g in range(4 * c, 4 * c + 4):
                nc.sync.dma_start(out=h3t4[64:72, (g - 4 * c) * 512:(g - 4 * c + 1) * 512],
                                  in_=qfb[g * 8:(g + 1) * 8, :])
            amlp = hdp.tile([128, CB], BF16, tag="amlp", name="amlp")
            fmlp = hdp.tile([128, CB], BF16, tag="fmlp", name="fmlp")
            lb = hdp.tile([3, CB], F32, tag="lb", name="lb")
            ob = hdp.tile([3, CB], F32, tag="ob", name="ob")

            for s in range(NSLC):
                sl = slice(s * NSL, (s + 1) * NSL)

                # conv1: K=108, M=576
                for mj, (m0_, m1_) in enumerate(m1p):
                    r = m1_ - m0_
                    ps = psp.tile([128, NSL], F32, tag="ps", name="ps")
                    nc.tensor.matmul(ps[:r], t1[:, m0_:m1_], xc[:, sl],
                                     start=True, stop=True)
                    nc.scalar.activation(h1[mj][:, sl], ps[:r], AF.Relu,
                                         bias=b1[:r, mj:mj + 1])

                # conv2: K=576 (5 subtiles), M=1152 (9 tiles), banded
                for mj, (m0_, m1_) in enumerate(m2p):
                    r = m1_ - m0_
                    ks = blocks2[mj]
                    ps = psp.tile([128, NSL], F32, tag="ps", name="ps")
                    for i, ki in enumerate(ks):
                        nc.tensor.matmul(ps[:r], t2s[ki][:, m0_:m1_],
                                         h1[ki][:, sl],
                                         start=(i == 0), stop=(i == len(ks) - 1))
                    if mj < 5:
                        nc.scalar.activation(h2[mj][:, sl], ps[:r], AF.Relu,
                                             bias=b2[:r, mj:mj + 1])
                    else:
                        nc.vector.tensor_scalar(h2[mj][:, sl], ps[:r],
                                                b2[:r, mj:mj + 1], 0.0,
                                                ALU.add, ALU.max)

            # parity gather (DMA) + maxpool, two rounds to halve tile count
            for rnd in range(2):
                par = [prp.tile([r, CB], BF16, tag=f"par_{p}_{t}",
                                name=f"par_{rnd}_{p}_{t}")
                       for p in range(2) for t, r in enumerate((128, 128, 32))]
                for st, sr, p, dt_, dr in gather:
                    if p // 2 == rnd:
                        nc.sync.dma_start(
                            out=par[(p % 2) * 3 + dt_][dr:dr + 32, :],
                            in_=h2[st][sr:sr + 32, :])
                for t in range(3):
                    r = (128, 128, 32)[t]
                    if rnd == 0:
                        nc.vector.tensor_max(pool[t][:r], par[t][:r], par[3 + t][:r])
                    else:
                        nc.vector.tensor_max(par[t][:r], par[t][:r], par[3 + t][:r])
                        nc.vector.tensor_max(pool[t][:r], pool[t][:r], par[t][:r])

            for s in range(NSLC):
                sl = slice(s * NSL, (s + 1) * NSL)

                # conv3: K=288 (3 subtiles), M=576 (5 tiles)
                for mj, (m0_, m1_) in enumerate(m3p):
                    r = m1_ - m0_
                    ks = blocks3[mj]
                    ps = psp.tile([128, NSL], F32, tag="ps", name="ps")
                    for i, ki in enumerate(ks):
                        nc.tensor.matmul(ps[:r], t3s[ki][:, m0_:m1_],
                                         pool[ki][:, sl],
                                         start=(i == 0), stop=(i == len(ks) - 1))
                    dst = h3[mj][:, sl] if mj < 4 else h3t4[0:64, sl]
                    nc.vector.tensor_scalar(dst, ps[:r], b3[:r, mj:mj + 1], 0.0,
                                            ALU.add, ALU.max)

                # mlp layer 1: K=584 (5 subtiles), M=192 (pt 128 | cf 64)
                rhs5 = [h3[0][:, sl], h3[1][:, sl], h3[2][:, sl], h3[3][:, sl],
                        h3t4[:, sl]]
                for mj, (m0_, m1_) in enumerate(((0, 128), (128, 192))):
                    r = m1_ - m0_
                    ps = psp.tile([128, NSL], F32, tag="ps", name="ps")
                    for i in range(5):
                        nc.tensor.matmul(ps[:r], w1s[i][:, m0_:m1_], rhs5[i],
                                         start=(i == 0), stop=(i == 4))
                    dst = amlp[:, sl] if mj == 0 else fmlp[64:128, sl]
                    nc.vector.tensor_scalar(dst, ps[:r], bm1[:r, mj:mj + 1], 0.0,
                                            ALU.add, ALU.max)

                # mlp layer 2 (pt head): K=128, M=64 -> F rows 0:64
                ps = psp.tile([128, NSL], F32, tag="ps", name="ps")
                nc.tensor.matmul(ps[:64], w2, amlp[:, sl], start=True, stop=True)
                nc.vector.tensor_scalar(fmlp[0:64, sl], ps[:64],
                                        bm2[:64, 0:1], 0.0, ALU.add, ALU.max)

                # final: [l0,l1,conf] then differences then tanh-sigmoid
                ph = psh.tile([3, NSL], F32, tag="psh", name="ph")
                nc.tensor.matmul(ph, w3, fmlp[:, sl], start=True, stop=True)
                nc.scalar.activation(lb[:, sl], ph, AF.Identity, bias=bh[:, 0:1])
                pd = psh.tile([3, NSL], F32, tag="psh", name="pd")
                nc.tensor.matmul(pd, smat, lb[:, sl], start=True, stop=True)
                nc.scalar.activation(ob[:, sl], pd, AF.Tanh, bias=zc[:3], scale=0.5)
                nc.vector.tensor_scalar(ob[:, sl], ob[:, sl], 0.5, 0.5,
                                        ALU.mult, ALU.add)

            nc.sync.dma_start(out=out_d[:, c0:c0 + CB], in_=ob)

    nc.compile()
    return nc


def _prep_host(inputs):
    conv1_w = np.asarray(inputs["conv1_w"], np.float32)
    conv2_w = np.asarray(inputs["conv2_w"], np.float32)
    conv3_w = np.asarray(inputs["conv3_w"], np.float32)
    T1, T2, T3 = _build_conv_maps(conv1_w, conv2_w, conv3_w)

    _cache["blocks2"] = _nonzero_blocks(T2, _parts(576), _parts(1152))
    _cache["blocks3"] = _nonzero_blocks(T3, _parts(288), _parts(576))

    # MLP weights, conv rows permuted into my pos-major H3 ordering
    pt_w1 = np.asarray(inputs["pt_w1"], np.float32)
    cf_w1 = np.asarray(inputs["cf_w1"], np.float32)
    perm = np.empty(584, np.int64)
    for pos in range(9):
        for co in range(64):
            perm[pos * 64 + co] = co * 9 + pos
    perm[576:] = np.arange(576, 584)
    W1 = np.concatenate([pt_w1[perm], cf_w1[perm]], axis=1)  # [584, 192]

    W3 = np.zeros((128, 3), np.float32)
    W3[0:64, 0:2] = np.asarray(inputs["pt_w3"], np.float32)
    W3[64:128, 2] = np.asarray(inputs["cf_w2"], np.float32)[:, 0]

    S = np.zeros((3, 3), np.float32)
    S[:, 0] = (1, -1, 0)
    S[:, 1] = (-1, 1, 0)
    S[:, 2] = (0, 0, 1)

    def pack_bias(b, per, ntile, rows):
        full = np.tile(np.asarray(b, np.float32), rows // per * ntile)[:rows * ntile]
        out = np.zeros((128, ntile), np.float32)
        for m in range(ntile):
            seg = full[m * 128:(m + 1) * 128] if rows * ntile - m * 128 >= 128 \
                else np.pad(full[m * 128:], (0, 128 - (rows * ntile - m * 128)))
            out[:len(seg), m] = seg
        return out

    def pack_bias2(bvec, total, ntile):
        full = np.zeros(ntile * 128, np.float32)
        full[:total] = bvec
        return full.reshape(ntile, 128).T.copy()

    b1 = pack_bias2(np.tile(np.asarray(inputs["conv1_b"], np.float32), 36), 576, 5)
    b2 = pack_bias2(np.tile(np.asarray(inputs["conv2_b"], np.float32), 36), 1152, 9)
    b3 = pack_bias2(np.tile(np.asarray(inputs["conv3_b"], np.float32), 9), 576, 5)
    bm1 = pack_bias2(np.concatenate([np.asarray(inputs["pt_b1"], np.float32),
                                     np.asarray(inputs["cf_b1"], np.float32)]), 192, 2)
    bm2 = pack_bias2(np.asarray(inputs["pt_b2"], np.float32), 64, 1)
    bh = np.concatenate([np.asarray(inputs["pt_b3"], np.float32),
                         np.asarray(inputs["cf_b2"], np.float32)]).reshape(3, 1)

    qp = np.asarray(inputs["quantum_params"], np.float32)  # [3,8,3]
    rot = np.zeros((128, 9), np.float32)
    for g in range(16):
        for q in range(8):
            for l in range(3):
                for i in range(3):
                    rot[q + 8 * g, l * 3 + i] = qp[l, q, i]

    shared = {
        "t1d": T1.astype(nbf), "t2d": T2.astype(nbf), "t3d": T3.astype(nbf),
        "w1d": W1.astype(nbf), "w2pd": np.asarray(inputs["pt_w2"], np.float32).astype(nbf),
        "w3cd": W3.astype(nbf), "Sd": S,
        "b1t": b1, "b2t": b2, "b3t": b3, "bm1": bm1, "bm2": bm2, "bh": bh,
        "rot": rot,
    }

    board = np.asarray(inputs["board_state"], np.float32).reshape(B, 108)
    in_maps = []
    for c in range(NCORES):
        bx = board[c * BC:(c + 1) * BC]          # [8192, 108]
        xq = bx[:, :NQ]                           # [8192, 8]
        xqn = np.roll(xq, -1, axis=1)
        m = dict(shared)
        m["xT"] = np.ascontiguousarray(bx.T).astype(nbf)
        m["qx"] = np.ascontiguousarray(
            xq.reshape(16, BC // 16, 8).transpose(0, 2, 1).reshape(128, BC // 16))
        m["qxn"] = np.ascontiguousarray(
            xqn.reshape(16, BC // 16, 8).transpose(0, 2, 1).reshape(128, BC // 16))
        in_maps.append(m)
    return in_maps


def kernel(**inputs):
    in_maps = _prep_host(inputs)
    if "nc" not in _cache:
        _cache["nc"] = _build_program()
    import os
    trace = os.environ.get("BASS_TRACE", "0") == "1"
    res = run_bass_kernel_spmd(_cache["nc"], in_maps, core_ids=list(range(NCORES)),
                               trace=trace)
    if res.exec_time_ns is not None:
        print(f"HW exec time: {res.exec_time_ns} ns")
        if res.instructions_and_trace is not None:
            print("trace:", res.instructions_and_trace[1])
    out = np.empty((B, 3), np.float32)
    for c in range(NCORES):
        out[c * BC:(c + 1) * BC] = res.results[c]["out"].T
    return out


if __name__ == "__main__":
    rng = np.random.default_rng(0)
    fake = {
        "board_state": rng.standard_normal((B, 3, 6, 6), dtype=np.float32),
        "target_positions": np.zeros((4, 2), np.int64),
        "conv1_w": rng.standard_normal((16, 3, 3, 3), dtype=np.float32) * 0.1,
        "conv1_b": rng.standard_normal(16, dtype=np.float32) * 0.1,
        "conv2_w": rng.standard_normal((32, 16, 3, 3), dtype=np.float32) * 0.05,
        "conv2_b": rng.standard_normal(32, dtype=np.float32) * 0.1,
        "conv3_w": rng.standard_normal((64, 32, 3, 3), dtype=np.float32) * 0.05,
        "conv3_b": rng.standard_normal(64, dtype=np.float32) * 0.1,
        "quantum_params": rng.standard_normal((3, 8, 3), dtype=np.float32),
        "pt_w1": rng.standard_normal((584, 128), dtype=np.float32) * 0.04,
        "pt_b1": rng.standard_normal(128, dtype=np.float32) * 0.04,
        "pt_w2": rng.standard_normal((128, 64), dtype=np.float32) * 0.09,
        "pt_b2": rng.standard_normal(64, dtype=np.float32) * 0.09,
        "pt_w3": rng.standard_normal((64, 2), dtype=np.float32) * 0.125,
        "pt_b3": rng.standard_normal(2, dtype=np.float32) * 0.125,
        "cf_w1": rng.standard_normal((584, 64), dtype=np.float32) * 0.04,
        "cf_b1": rng.standard_normal(64, dtype=np.float32) * 0.04,
        "cf_w2": rng.standard_normal((64, 1), dtype=np.float32) * 0.125,
        "cf_b2": rng.standard_normal(1, dtype=np.float32) * 0.125,
    }
    o = kernel(**fake)
    print(o.shape, o[:2])
